# revision 18
# baseline (speedup 1.0000x reference)
"""Sharded causal-attention kernel for 8 trn2 NeuronCores.

DP over batch (2) x TP over head groups (4 heads/core). Each core: qkv projection
(its heads) + RoPE + causal SDPA (scores kept transposed; softmax denominator via a
ones-column in the PV matmul) + its 256-row slice of the o_proj contraction, returning
a transposed partial [HID, S]; the host sums 4 partials per batch. bf16 matmuls,
fp32 PSUM accumulation.
"""

import sys

sys.path.insert(0, "/opt/trn_rl_repo")

from contextlib import ExitStack

import numpy as np
import ml_dtypes

import concourse.bass as bass
import concourse.mybir as mybir
import concourse.tile as tile
from concourse import bacc

FP = mybir.dt.float32
BF = mybir.dt.bfloat16
EXP = mybir.ActivationFunctionType.Exp

B, S, HID = 2, 2048, 1024
H, D = 16, 64
QC = 512
KT = 128
NQC = S // QC
NKT = S // KT
KHID = HID // 128


def build_program(debug_outputs=False):
    nc = bacc.Bacc("TRN2", target_bir_lowering=False, debug=False, num_devices=8)

    hsT = nc.dram_tensor("hsT", [HID, S], BF, kind="ExternalInput").ap()
    wqkT = nc.dram_tensor("wqkT", [HID, 512], BF, kind="ExternalInput").ap()
    wvT = nc.dram_tensor("wvT", [HID, 256], BF, kind="ExternalInput").ap()
    woT = nc.dram_tensor("woT", [256, HID], BF, kind="ExternalInput").ap()
    cos2T = nc.dram_tensor("cos2T", [128, S], BF, kind="ExternalInput").ap()
    ssin2T = nc.dram_tensor("ssin2T", [128, S], BF, kind="ExternalInput").ap()
    maskD = nc.dram_tensor("maskD", [128, 256], BF, kind="ExternalInput").ap()
    outT = nc.dram_tensor("outT", [HID, S], FP, kind="ExternalOutput").ap()
    dbg = None
    if debug_outputs:
        dbg = {
            "dbg_qk": nc.dram_tensor("dbg_qk", [512, S], BF, kind="ExternalOutput").ap(),
            "dbg_v": nc.dram_tensor("dbg_v", [128, NKT * 4 * 65], BF, kind="ExternalOutput").ap(),
            "dbg_att": nc.dram_tensor("dbg_att", [256, S], BF, kind="ExternalOutput").ap(),
        }

    with tile.TileContext(nc) as tc:
        build_tile_program(tc, hsT, wqkT, wvT, woT, cos2T, ssin2T, maskD, outT, dbg)
    nc.compile()
    return nc


def build_tile_program(tc, hsT, wqkT, wvT, woT, cos2T, ssin2T, maskD, outT, dbg=None):
    nc = tc.nc
    with ExitStack() as ctx:
        const = ctx.enter_context(tc.tile_pool(name="const", bufs=1))
        persist = ctx.enter_context(tc.tile_pool(name="persist", bufs=1))
        work = ctx.enter_context(tc.tile_pool(name="work", bufs=3))
        posbp = ctx.enter_context(tc.tile_pool(name="posbp", bufs=12))
        expp = ctx.enter_context(tc.tile_pool(name="expp", bufs=8))
        small = ctx.enter_context(tc.tile_pool(name="small", bufs=3))
        ps_main = ctx.enter_context(tc.tile_pool(name="ps_main", bufs=4, space="PSUM"))
        ps_sc = ctx.enter_context(tc.tile_pool(name="ps_sc", bufs=2, space="PSUM"))

        # ---- inputs to SBUF (one DMA instruction per tensor / hs chunk) ----
        wqk_sb = const.tile([128, KHID, 512], BF, name="wqk_sb")
        nc.scalar.dma_start(wqk_sb[:, 0:4, :],
                            wqkT[0:512, :].rearrange("(k p) m -> p k m", p=128))
        hs_sb = const.tile([128, KHID, S], BF, name="hs_sb")

        def load_hs_chunk(t):
            csl = slice(t * QC, (t + 1) * QC)
            nc.sync.dma_start(
                hs_sb[:, :, csl],
                hsT[:, csl].rearrange("(k p) s -> p k s", p=128),
            )

        nc.sync.dma_start(hs_sb[:, 0:4, 0:QC],
                          hsT[0:512, 0:QC].rearrange("(k p) s -> p k s", p=128))
        nc.gpsimd.dma_start(wqk_sb[:, 4:8, :],
                            wqkT[512:1024, :].rearrange("(k p) m -> p k m", p=128))
        nc.sync.dma_start(hs_sb[:, 4:8, 0:QC],
                          hsT[512:1024, 0:QC].rearrange("(k p) s -> p k s", p=128))
        cos_sb = const.tile([128, S], BF, name="cos_sb")
        nc.sync.dma_start(cos_sb[:], cos2T[:])
        load_hs_chunk(1)
        ssin_sb = const.tile([128, S], BF, name="ssin_sb")
        nc.sync.dma_start(ssin_sb[:], ssin2T[:])
        tri_sb = const.tile([128, 2, 128], BF, name="tri_sb")
        nc.sync.dma_start(tri_sb[:], maskD.rearrange("p (r c) -> p r c", r=2))
        load_hs_chunk(2)
        load_hs_chunk(3)
        wv_sb = const.tile([128, KHID, 256], BF, name="wv_sb")
        nc.sync.dma_start(wv_sb[:], wvT.rearrange("(k p) m -> p k m", p=128))
        wo_sb = const.tile([128, 2, HID], BF, name="wo_sb")
        nc.sync.dma_start(wo_sb[:], woT.rearrange("(k p) m -> p k m", p=128))

        qkT = persist.tile([128, 4, S], BF, name="qkT")
        l_tiles = persist.tile([64, 8, QC], FP, name="l_tiles")
        nc.vector.memset(l_tiles[:], 1.0)
        v_sb = persist.tile([128, NKT, 4 * 65], BF, name="v_sb2")
        att_sb = persist.tile([128, 2, S], BF, name="att_sb2")
        nc.vector.memset(
            v_sb.rearrange("p t (h c) -> p t h c", c=65)[:, :, :, 64:65], 1.0
        )

        def proj_chunk(rb, t):
            csl = slice(t * QC, (t + 1) * QC)
            ps = ps_main.tile([128, QC], FP, name="ps_qk", tag="ps")
            for kk in range(KHID):
                nc.tensor.matmul(
                    ps[:],
                    wqk_sb[:, kk, rb * 128:(rb + 1) * 128],
                    hs_sb[:, kk, csl],
                    start=(kk == 0),
                    stop=(kk == KHID - 1),
                )
            x = work.tile([128, QC], BF, name="x_rope", tag="xrope")
            if rb in (1, 3):
                nc.vector.tensor_copy(x[:], ps[:])
            else:
                nc.scalar.copy(x[:], ps[:])
            xs = work.tile([128, QC], BF, name="xs_rope", tag="xsrope")
            for q in range(4):
                src = (q // 2) * 64 + (32 if q % 2 == 0 else 0)
                dst = (q // 2) * 64 + (0 if q % 2 == 0 else 32)
                nc.vector.tensor_copy(xs[dst:dst + 32, :], x[src:src + 32, :])
            t1 = work.tile([128, QC], BF, name="t1_rope", tag="t1rope")
            nc.vector.tensor_mul(t1[:], x[:], cos_sb[:, csl])
            nc.vector.tensor_mul(xs[:], xs[:], ssin_sb[:, csl])
            nc.vector.tensor_add(qkT[:, rb, csl], t1[:], xs[:])

        def v_proj(tt):
            psv = ps_main.tile([128, 256], FP, name="ps_v", tag="ps")
            for kk in range(KHID):
                nc.tensor.matmul(
                    psv[:],
                    hs_sb[:, kk, tt * 128:(tt + 1) * 128],
                    wv_sb[:, kk, :],
                    start=(kk == 0),
                    stop=(kk == KHID - 1),
                )
            nc.vector.tensor_copy(
                v_sb[:, tt, :].rearrange("p (h c) -> p h c", c=65)[:, :, 0:64],
                psv[:].rearrange("p (h c) -> p h c", c=64),
            )

        def attention_unit(pair, qi, l_pair):
            """scores^T -> exp -> PV for heads (2*pair, 2*pair+1).

            Returns (po_sb0, po_sb1); denominators land in l_pair rows 0 and 32."""
            qsl = slice(qi * QC, (qi + 1) * QC)
            nki = 4 * qi + 4
            po0 = ps_main.tile([65, QC], FP, name="po0", tag="ps")
            po1 = ps_main.tile([65, QC], FP, name="po1", tag="ps")
            for ki in range(nki):
                ksl = slice(ki * KT, (ki + 1) * KT)
                psc = ps_sc.tile([128, 2, QC], FP, name="psc", tag="sc")
                nc.tensor.matmul(
                    psc[:, 0, :], qkT[0:64, 2 + pair, ksl], qkT[0:64, pair, qsl],
                    start=True, stop=True,
                )
                nc.tensor.matmul(
                    psc[:, 1, :], qkT[64:128, 2 + pair, ksl], qkT[64:128, pair, qsl],
                    start=True, stop=True,
                )
                e = expp.tile([128, 2, QC], BF, name="e", tag="exp")
                j = ki - 4 * qi
                lo = 0 if j < 0 else 128 * j  # first live q column in this chunk
                nc.scalar.activation(
                    e[:, :, lo:QC], psc[:, :, lo:QC], EXP, scale=0.125
                )
                if j >= 0:
                    nc.vector.tensor_mul(
                        e[:, :, lo:lo + 128], e[:, :, lo:lo + 128], tri_sb[:]
                    )
                h0 = 2 * pair
                h1 = 2 * pair + 1
                nc.tensor.matmul(
                    po0[:, lo:QC], v_sb[:, ki, h0 * 65:(h0 + 1) * 65], e[:, 0, lo:QC],
                    start=(ki == 0), stop=(ki == nki - 1),
                )
                nc.tensor.matmul(
                    po1[:, lo:QC], v_sb[:, ki, h1 * 65:(h1 + 1) * 65], e[:, 1, lo:QC],
                    start=(ki == 0), stop=(ki == nki - 1),
                )
            # free PSUM fast: numerators to SBUF bf16, denominator rows to l_all
            po_sb0 = posbp.tile([64, QC], BF, name="po_sb0", tag="posb")
            po_sb1 = posbp.tile([64, QC], BF, name="po_sb1", tag="posb")
            nc.vector.tensor_copy(po_sb0[:], po0[0:64, :])
            nc.vector.tensor_copy(po_sb1[:], po1[0:64, :])
            nc.vector.tensor_copy(l_pair[0:1, :], po0[64:65, :])
            nc.vector.tensor_copy(l_pair[32:33, :], po1[64:65, :])
            return po_sb0, po_sb1

        def division(pair, qi, l_pair, po_sb0, po_sb1):
            qsl = slice(qi * QC, (qi + 1) * QC)
            rl = small.tile([64, QC], FP, name="rl", tag="rl", bufs=4)
            scr = small.tile([64, QC], FP, name="scr", tag="scr", bufs=3)
            nc.vector.reciprocal_approx_accurate(out=rl[:], in_=l_pair[:], scratch=scr[:])
            for sub, posb in enumerate([po_sb0, po_sb1]):
                if sub == 0:
                    src = rl[0:1, :]
                else:
                    rlrow = small.tile([1, QC], FP, name="rlrow", tag="rlrow", bufs=4)
                    nc.vector.tensor_copy(rlrow[:], rl[32:33, :])
                    src = rlrow[:]
                rb_ = small.tile([64, QC], FP, name="rb_", tag="rbb", bufs=4)
                nc.gpsimd.partition_broadcast(rb_[:], src)
                nc.vector.tensor_mul(
                    att_sb[sub * 64:(sub + 1) * 64, pair, qsl], posb[:], rb_[:]
                )

        def oproj(qi, last=False):
            qsl = slice(qi * QC, (qi + 1) * QC)
            for half in range(2):
                ow = work.tile([128, 4, QC], FP, name="ow", tag="ow")
                for oi in range(4):
                    ot = half * 4 + oi
                    pw = ps_main.tile([128, QC], FP, name="pw", tag="ps")
                    for p in range(2):
                        nc.tensor.matmul(
                            pw[:],
                            wo_sb[:, p, ot * 128:(ot + 1) * 128],
                            att_sb[:, p, qsl],
                            start=(p == 0),
                            stop=(p == 1),
                        )
                    if last and ot % 2 == 0:
                        nc.vector.tensor_copy(ow[:, oi, :], pw[:])
                    else:
                        nc.scalar.copy(ow[:, oi, :], pw[:])
                nc.gpsimd.dma_start(
                    outT[half * 512:(half + 1) * 512, qsl].rearrange(
                        "(o p) s -> p o s", p=128),
                    ow[:],
                )

        # emission: pair0 projections up front; pair1 projections, v, attention
        # and (one chunk behind) o_proj interleaved per q chunk so the PE always
        # has ready fill work while ACT grinds through the exps.
        for t in range(NQC):
            proj_chunk(0, t)
            proj_chunk(2, t)
        qi_order = [1, 2, 3, 0]
        loaded = 0
        prev = None
        for qi in qi_order:
            while loaded <= qi:
                proj_chunk(1, loaded)
                proj_chunk(3, loaded)
                for tt in range(4 * loaded, 4 * loaded + 4):
                    v_proj(tt)
                loaded += 1
            l0 = l_tiles[:, 2 * qi, :]
            pa = attention_unit(0, qi, l0)
            division(0, qi, l0, *pa)
            l1 = l_tiles[:, 2 * qi + 1, :]
            pb = attention_unit(1, qi, l1)
            if prev is not None:
                oproj(prev)
            division(1, qi, l1, *pb)
            prev = qi
        oproj(prev, last=True)

        if dbg is not None:
            for rb in range(4):
                nc.sync.dma_start(dbg["dbg_qk"][rb * 128:(rb + 1) * 128, :], qkT[:, rb, :])
            nc.sync.dma_start(dbg["dbg_v"][:], v_sb.rearrange("p t c -> p (t c)"))
            for p in range(2):
                nc.sync.dma_start(dbg["dbg_att"][p * 128:(p + 1) * 128, :], att_sb[:, p, :])


# ---------- host-side shard preparation ----------

def make_core_inputs(hidden_states, cos, sin, w_qkv, w_o):
    """Returns list of 8 in_maps (numpy, bf16 where needed)."""
    bf = ml_dtypes.bfloat16
    hs = np.asarray(hidden_states, np.float32)
    cos = np.asarray(cos, np.float32)
    sin = np.asarray(sin, np.float32)
    w_qkv = np.asarray(w_qkv, np.float32)
    w_o = np.asarray(w_o, np.float32)

    cosT = cos.T
    sinT = sin.T
    cos2T = np.concatenate([cosT, cosT], 0).astype(bf)
    ssinT = np.concatenate([-sinT[0:32], sinT[32:64]], 0)
    ssin2T = np.concatenate([ssinT, ssinT], 0).astype(bf)

    kp = np.arange(128)[:, None]
    cc = np.arange(128)[None, :]
    tri = (kp <= cc).astype(bf)
    maskD = np.concatenate([tri, tri], axis=1)

    in_maps = []
    for c in range(8):
        b, g = divmod(c, 4)
        heads = range(4 * g, 4 * g + 4)
        hsT = np.ascontiguousarray(hs[b].T).astype(bf)
        wq = np.concatenate([w_qkv[h * 64:(h + 1) * 64] for h in heads], 0)
        wk = np.concatenate([w_qkv[HID + h * 64:HID + (h + 1) * 64] for h in heads], 0)
        wv = np.concatenate([w_qkv[2 * HID + h * 64:2 * HID + (h + 1) * 64] for h in heads], 0)
        wqkT = np.ascontiguousarray(np.concatenate([wq, wk], 0).T).astype(bf)
        wvT = np.ascontiguousarray(wv.T).astype(bf)
        woT = np.ascontiguousarray(
            np.concatenate([w_o[:, h * 64:(h + 1) * 64] for h in heads], 1).T
        ).astype(bf)
        in_maps.append({
            "hsT": hsT, "wqkT": wqkT, "wvT": wvT, "woT": woT,
            "cos2T": cos2T, "ssin2T": ssin2T, "maskD": maskD,
        })
    return in_maps


def unshard(outTs):
    out = np.zeros((B, S, HID), np.float32)
    for c, oT in enumerate(outTs):
        out[c // 4] += oT.T
    return out


# ---------- standalone kernel entry ----------

from concourse.bass_utils import run_bass_kernel_spmd

_CACHED_NC = None


def get_program():
    global _CACHED_NC
    if _CACHED_NC is None:
        _CACHED_NC = build_program()
    return _CACHED_NC


def run(inputs, trace=False):
    nc = get_program()
    in_maps = make_core_inputs(**inputs)
    res = run_bass_kernel_spmd(nc, in_maps, core_ids=list(range(8)), trace=trace)
    out = np.zeros((B, S, HID), np.float32)
    for c, r in enumerate(res.results):
        out[c // 4] += r["outT"].T
    return out, res


def kernel(**inputs):
    out, _ = run(inputs, trace=False)
    return out


# revision 19
# speedup vs baseline: 1.0163x; 1.0163x over previous
"""Sharded causal-attention kernel for 8 trn2 NeuronCores.

DP over batch (2) x TP over head groups (4 heads/core). Each core: qkv projection
(its heads) + RoPE + causal SDPA (scores kept transposed; softmax denominator via a
ones-column in the PV matmul) + its 256-row slice of the o_proj contraction, returning
a transposed partial [HID, S]; the host sums 4 partials per batch. bf16 matmuls,
fp32 PSUM accumulation.
"""

import sys

sys.path.insert(0, "/opt/trn_rl_repo")

from contextlib import ExitStack

import numpy as np
import ml_dtypes

import concourse.bass as bass
import concourse.mybir as mybir
import concourse.tile as tile
from concourse import bacc

FP = mybir.dt.float32
BF = mybir.dt.bfloat16
EXP = mybir.ActivationFunctionType.Exp

B, S, HID = 2, 2048, 1024
H, D = 16, 64
QC = 512
KT = 128
NQC = S // QC
NKT = S // KT
KHID = HID // 128


def build_program(debug_outputs=False):
    nc = bacc.Bacc("TRN2", target_bir_lowering=False, debug=False, num_devices=8)

    hsT = nc.dram_tensor("hsT", [HID, S], BF, kind="ExternalInput").ap()
    wqkT = nc.dram_tensor("wqkT", [HID, 512], BF, kind="ExternalInput").ap()
    wvT = nc.dram_tensor("wvT", [HID, 256], BF, kind="ExternalInput").ap()
    woT = nc.dram_tensor("woT", [256, HID], BF, kind="ExternalInput").ap()
    cos2T = nc.dram_tensor("cos2T", [128, S], BF, kind="ExternalInput").ap()
    ssin2T = nc.dram_tensor("ssin2T", [128, S], BF, kind="ExternalInput").ap()
    maskD = nc.dram_tensor("maskD", [128, 256], BF, kind="ExternalInput").ap()
    outT = nc.dram_tensor("outT", [HID, S], FP, kind="ExternalOutput").ap()
    dbg = None
    if debug_outputs:
        dbg = {
            "dbg_qk": nc.dram_tensor("dbg_qk", [512, S], BF, kind="ExternalOutput").ap(),
            "dbg_v": nc.dram_tensor("dbg_v", [128, NKT * 4 * 65], BF, kind="ExternalOutput").ap(),
            "dbg_att": nc.dram_tensor("dbg_att", [256, S], BF, kind="ExternalOutput").ap(),
        }

    with tile.TileContext(nc) as tc:
        build_tile_program(tc, hsT, wqkT, wvT, woT, cos2T, ssin2T, maskD, outT, dbg)
    nc.compile()
    return nc


def build_tile_program(tc, hsT, wqkT, wvT, woT, cos2T, ssin2T, maskD, outT, dbg=None):
    nc = tc.nc
    with ExitStack() as ctx:
        const = ctx.enter_context(tc.tile_pool(name="const", bufs=1))
        persist = ctx.enter_context(tc.tile_pool(name="persist", bufs=1))
        work = ctx.enter_context(tc.tile_pool(name="work", bufs=3))
        posbp = ctx.enter_context(tc.tile_pool(name="posbp", bufs=12))
        expp = ctx.enter_context(tc.tile_pool(name="expp", bufs=8))
        small = ctx.enter_context(tc.tile_pool(name="small", bufs=3))
        ps_main = ctx.enter_context(tc.tile_pool(name="ps_main", bufs=4, space="PSUM"))
        ps_sc = ctx.enter_context(tc.tile_pool(name="ps_sc", bufs=2, space="PSUM"))

        # ---- inputs to SBUF (one DMA instruction per tensor / hs chunk) ----
        wqk_sb = const.tile([128, KHID, 512], BF, name="wqk_sb")
        nc.scalar.dma_start(wqk_sb[:, 0:4, :],
                            wqkT[0:512, :].rearrange("(k p) m -> p k m", p=128))
        hs_sb = const.tile([128, KHID, S], BF, name="hs_sb")

        def load_hs_chunk(t):
            csl = slice(t * QC, (t + 1) * QC)
            nc.sync.dma_start(
                hs_sb[:, :, csl],
                hsT[:, csl].rearrange("(k p) s -> p k s", p=128),
            )

        nc.sync.dma_start(hs_sb[:, 0:4, 0:QC],
                          hsT[0:512, 0:QC].rearrange("(k p) s -> p k s", p=128))
        nc.gpsimd.dma_start(wqk_sb[:, 4:8, :],
                            wqkT[512:1024, :].rearrange("(k p) m -> p k m", p=128))
        nc.sync.dma_start(hs_sb[:, 4:8, 0:QC],
                          hsT[512:1024, 0:QC].rearrange("(k p) s -> p k s", p=128))
        cos_sb = const.tile([128, S], BF, name="cos_sb")
        nc.sync.dma_start(cos_sb[:], cos2T[:])
        load_hs_chunk(1)
        ssin_sb = const.tile([128, S], BF, name="ssin_sb")
        nc.sync.dma_start(ssin_sb[:], ssin2T[:])
        tri_sb = const.tile([128, 2, 128], BF, name="tri_sb")
        nc.sync.dma_start(tri_sb[:], maskD.rearrange("p (r c) -> p r c", r=2))
        load_hs_chunk(2)
        load_hs_chunk(3)
        wv_sb = const.tile([128, KHID, 256], BF, name="wv_sb")
        nc.sync.dma_start(wv_sb[:], wvT.rearrange("(k p) m -> p k m", p=128))
        wo_sb = const.tile([128, 2, HID], BF, name="wo_sb")
        nc.sync.dma_start(wo_sb[:], woT.rearrange("(k p) m -> p k m", p=128))

        qkT = persist.tile([128, 4, S], BF, name="qkT")
        l_tiles = persist.tile([64, 8, QC], FP, name="l_tiles")
        nc.vector.memset(l_tiles[:], 1.0)
        v_sb = persist.tile([128, NKT, 4 * 65], BF, name="v_sb2")
        att_sb = persist.tile([128, 2, S], BF, name="att_sb2")
        nc.vector.memset(
            v_sb.rearrange("p t (h c) -> p t h c", c=65)[:, :, :, 64:65], 1.0
        )

        def proj_chunk(rb, t):
            csl = slice(t * QC, (t + 1) * QC)
            ps = ps_main.tile([128, QC], FP, name="ps_qk", tag="ps")
            for kk in range(KHID):
                nc.tensor.matmul(
                    ps[:],
                    wqk_sb[:, kk, rb * 128:(rb + 1) * 128],
                    hs_sb[:, kk, csl],
                    start=(kk == 0),
                    stop=(kk == KHID - 1),
                )
            x = work.tile([128, QC], BF, name="x_rope", tag="xrope")
            if rb in (1, 3):
                nc.vector.tensor_copy(x[:], ps[:])
            else:
                nc.scalar.copy(x[:], ps[:])
            xs = work.tile([128, QC], BF, name="xs_rope", tag="xsrope")
            for q in range(4):
                src = (q // 2) * 64 + (32 if q % 2 == 0 else 0)
                dst = (q // 2) * 64 + (0 if q % 2 == 0 else 32)
                nc.vector.tensor_copy(xs[dst:dst + 32, :], x[src:src + 32, :])
            t1 = work.tile([128, QC], BF, name="t1_rope", tag="t1rope")
            nc.vector.tensor_mul(t1[:], x[:], cos_sb[:, csl])
            nc.vector.tensor_mul(xs[:], xs[:], ssin_sb[:, csl])
            nc.vector.tensor_add(qkT[:, rb, csl], t1[:], xs[:])

        def v_proj(tt):
            psv = ps_main.tile([128, 256], FP, name="ps_v", tag="ps")
            for kk in range(KHID):
                nc.tensor.matmul(
                    psv[:],
                    hs_sb[:, kk, tt * 128:(tt + 1) * 128],
                    wv_sb[:, kk, :],
                    start=(kk == 0),
                    stop=(kk == KHID - 1),
                )
            nc.vector.tensor_copy(
                v_sb[:, tt, :].rearrange("p (h c) -> p h c", c=65)[:, :, 0:64],
                psv[:].rearrange("p (h c) -> p h c", c=64),
            )

        def attention_unit(pair, qi, l_pair):
            """scores^T -> exp -> PV for heads (2*pair, 2*pair+1).

            Returns (po_sb0, po_sb1); denominators land in l_pair rows 0 and 32."""
            qsl = slice(qi * QC, (qi + 1) * QC)
            nki = 4 * qi + 4
            po0 = ps_main.tile([65, QC], FP, name="po0", tag="ps")
            po1 = ps_main.tile([65, QC], FP, name="po1", tag="ps")
            for ki in range(nki):
                ksl = slice(ki * KT, (ki + 1) * KT)
                psc = ps_sc.tile([128, 2, QC], FP, name="psc", tag="sc")
                nc.tensor.matmul(
                    psc[:, 0, :], qkT[0:64, 2 + pair, ksl], qkT[0:64, pair, qsl],
                    start=True, stop=True,
                )
                nc.tensor.matmul(
                    psc[:, 1, :], qkT[64:128, 2 + pair, ksl], qkT[64:128, pair, qsl],
                    start=True, stop=True,
                )
                e = expp.tile([128, 2, QC], BF, name="e", tag="exp")
                j = ki - 4 * qi
                lo = 0 if j < 0 else 128 * j  # first live q column in this chunk
                nc.scalar.activation(
                    e[:, :, lo:QC], psc[:, :, lo:QC], EXP, scale=0.125
                )
                if j >= 0:
                    nc.vector.tensor_mul(
                        e[:, :, lo:lo + 128], e[:, :, lo:lo + 128], tri_sb[:]
                    )
                h0 = 2 * pair
                h1 = 2 * pair + 1
                nc.tensor.matmul(
                    po0[:, lo:QC], v_sb[:, ki, h0 * 65:(h0 + 1) * 65], e[:, 0, lo:QC],
                    start=(ki == 0), stop=(ki == nki - 1),
                )
                nc.tensor.matmul(
                    po1[:, lo:QC], v_sb[:, ki, h1 * 65:(h1 + 1) * 65], e[:, 1, lo:QC],
                    start=(ki == 0), stop=(ki == nki - 1),
                )
            # free PSUM fast: numerators to SBUF bf16, denominator rows to l_all
            po_sb0 = posbp.tile([64, QC], BF, name="po_sb0", tag="posb")
            po_sb1 = posbp.tile([64, QC], BF, name="po_sb1", tag="posb")
            nc.vector.tensor_copy(po_sb0[:], po0[0:64, :])
            nc.vector.tensor_copy(po_sb1[:], po1[0:64, :])
            nc.vector.tensor_copy(l_pair[0:1, :], po0[64:65, :])
            nc.vector.tensor_copy(l_pair[32:33, :], po1[64:65, :])
            return po_sb0, po_sb1

        def division(pair, qi, l_pair, po_sb0, po_sb1):
            qsl = slice(qi * QC, (qi + 1) * QC)
            rl = small.tile([64, QC], FP, name="rl", tag="rl", bufs=4)
            scr = small.tile([64, QC], FP, name="scr", tag="scr", bufs=3)
            nc.vector.reciprocal_approx_accurate(out=rl[:], in_=l_pair[:], scratch=scr[:])
            for sub, posb in enumerate([po_sb0, po_sb1]):
                if sub == 0:
                    src = rl[0:1, :]
                else:
                    rlrow = small.tile([1, QC], FP, name="rlrow", tag="rlrow", bufs=4)
                    nc.vector.tensor_copy(rlrow[:], rl[32:33, :])
                    src = rlrow[:]
                rb_ = small.tile([64, QC], FP, name="rb_", tag="rbb", bufs=4)
                nc.gpsimd.partition_broadcast(rb_[:], src)
                nc.vector.tensor_mul(
                    att_sb[sub * 64:(sub + 1) * 64, pair, qsl], posb[:], rb_[:]
                )

        def oproj(qi, last=False):
            qsl = slice(qi * QC, (qi + 1) * QC)
            for half in range(2):
                ow = work.tile([128, 4, QC], FP, name="ow", tag="ow")
                for oi in range(4):
                    ot = half * 4 + oi
                    pw = ps_main.tile([128, QC], FP, name="pw", tag="ps")
                    for p in range(2):
                        nc.tensor.matmul(
                            pw[:],
                            wo_sb[:, p, ot * 128:(ot + 1) * 128],
                            att_sb[:, p, qsl],
                            start=(p == 0),
                            stop=(p == 1),
                        )
                    nc.vector.tensor_copy(ow[:, oi, :], pw[:])
                nc.gpsimd.dma_start(
                    outT[half * 512:(half + 1) * 512, qsl].rearrange(
                        "(o p) s -> p o s", p=128),
                    ow[:],
                )

        # emission: pair0 projections up front; pair1 projections, v, attention
        # and (one chunk behind) o_proj interleaved per q chunk so the PE always
        # has ready fill work while ACT grinds through the exps.
        for t in range(NQC):
            proj_chunk(0, t)
            proj_chunk(2, t)
        qi_order = [1, 2, 3, 0]
        loaded = 0
        prev = None
        for qi in qi_order:
            while loaded <= qi:
                proj_chunk(1, loaded)
                proj_chunk(3, loaded)
                for tt in range(4 * loaded, 4 * loaded + 4):
                    v_proj(tt)
                loaded += 1
            l0 = l_tiles[:, 2 * qi, :]
            pa = attention_unit(0, qi, l0)
            division(0, qi, l0, *pa)
            l1 = l_tiles[:, 2 * qi + 1, :]
            pb = attention_unit(1, qi, l1)
            if prev is not None:
                oproj(prev)
            division(1, qi, l1, *pb)
            prev = qi
        oproj(prev, last=True)

        if dbg is not None:
            for rb in range(4):
                nc.sync.dma_start(dbg["dbg_qk"][rb * 128:(rb + 1) * 128, :], qkT[:, rb, :])
            nc.sync.dma_start(dbg["dbg_v"][:], v_sb.rearrange("p t c -> p (t c)"))
            for p in range(2):
                nc.sync.dma_start(dbg["dbg_att"][p * 128:(p + 1) * 128, :], att_sb[:, p, :])


# ---------- host-side shard preparation ----------

def make_core_inputs(hidden_states, cos, sin, w_qkv, w_o):
    """Returns list of 8 in_maps (numpy, bf16 where needed)."""
    bf = ml_dtypes.bfloat16
    hs = np.asarray(hidden_states, np.float32)
    cos = np.asarray(cos, np.float32)
    sin = np.asarray(sin, np.float32)
    w_qkv = np.asarray(w_qkv, np.float32)
    w_o = np.asarray(w_o, np.float32)

    cosT = cos.T
    sinT = sin.T
    cos2T = np.concatenate([cosT, cosT], 0).astype(bf)
    ssinT = np.concatenate([-sinT[0:32], sinT[32:64]], 0)
    ssin2T = np.concatenate([ssinT, ssinT], 0).astype(bf)

    kp = np.arange(128)[:, None]
    cc = np.arange(128)[None, :]
    tri = (kp <= cc).astype(bf)
    maskD = np.concatenate([tri, tri], axis=1)

    in_maps = []
    for c in range(8):
        b, g = divmod(c, 4)
        heads = range(4 * g, 4 * g + 4)
        hsT = np.ascontiguousarray(hs[b].T).astype(bf)
        wq = np.concatenate([w_qkv[h * 64:(h + 1) * 64] for h in heads], 0)
        wk = np.concatenate([w_qkv[HID + h * 64:HID + (h + 1) * 64] for h in heads], 0)
        wv = np.concatenate([w_qkv[2 * HID + h * 64:2 * HID + (h + 1) * 64] for h in heads], 0)
        wqkT = np.ascontiguousarray(np.concatenate([wq, wk], 0).T).astype(bf)
        wvT = np.ascontiguousarray(wv.T).astype(bf)
        woT = np.ascontiguousarray(
            np.concatenate([w_o[:, h * 64:(h + 1) * 64] for h in heads], 1).T
        ).astype(bf)
        in_maps.append({
            "hsT": hsT, "wqkT": wqkT, "wvT": wvT, "woT": woT,
            "cos2T": cos2T, "ssin2T": ssin2T, "maskD": maskD,
        })
    return in_maps


def unshard(outTs):
    out = np.zeros((B, S, HID), np.float32)
    for c, oT in enumerate(outTs):
        out[c // 4] += oT.T
    return out


# ---------- standalone kernel entry ----------

from concourse.bass_utils import run_bass_kernel_spmd

_CACHED_NC = None


def get_program():
    global _CACHED_NC
    if _CACHED_NC is None:
        _CACHED_NC = build_program()
    return _CACHED_NC


def run(inputs, trace=False):
    nc = get_program()
    in_maps = make_core_inputs(**inputs)
    res = run_bass_kernel_spmd(nc, in_maps, core_ids=list(range(8)), trace=trace)
    out = np.zeros((B, S, HID), np.float32)
    for c, r in enumerate(res.results):
        out[c // 4] += r["outT"].T
    return out, res


def kernel(**inputs):
    out, _ = run(inputs, trace=False)
    return out


# revision 20
# speedup vs baseline: 1.0330x; 1.0165x over previous
"""Sharded causal-attention kernel for 8 trn2 NeuronCores.

DP over batch (2) x TP over head groups (4 heads/core). Each core: qkv projection
(its heads) + RoPE + causal SDPA (scores kept transposed; softmax denominator via a
ones-column in the PV matmul) + its 256-row slice of the o_proj contraction, returning
a transposed partial [HID, S]; the host sums 4 partials per batch. bf16 matmuls,
fp32 PSUM accumulation.
"""

import sys

sys.path.insert(0, "/opt/trn_rl_repo")

from contextlib import ExitStack

import numpy as np
import ml_dtypes

import concourse.bass as bass
import concourse.mybir as mybir
import concourse.tile as tile
from concourse import bacc

FP = mybir.dt.float32
BF = mybir.dt.bfloat16
EXP = mybir.ActivationFunctionType.Exp

B, S, HID = 2, 2048, 1024
H, D = 16, 64
QC = 512
KT = 128
NQC = S // QC
NKT = S // KT
KHID = HID // 128


def build_program(debug_outputs=False):
    nc = bacc.Bacc("TRN2", target_bir_lowering=False, debug=False, num_devices=8)

    hsT = nc.dram_tensor("hsT", [HID, S], BF, kind="ExternalInput").ap()
    wqkT = nc.dram_tensor("wqkT", [HID, 512], BF, kind="ExternalInput").ap()
    wvT = nc.dram_tensor("wvT", [HID, 256], BF, kind="ExternalInput").ap()
    woT = nc.dram_tensor("woT", [256, HID], BF, kind="ExternalInput").ap()
    cos2T = nc.dram_tensor("cos2T", [128, S], BF, kind="ExternalInput").ap()
    ssin2T = nc.dram_tensor("ssin2T", [128, S], BF, kind="ExternalInput").ap()
    maskD = nc.dram_tensor("maskD", [128, 256], BF, kind="ExternalInput").ap()
    outT = nc.dram_tensor("outT", [HID, S], BF, kind="ExternalOutput").ap()
    dbg = None
    if debug_outputs:
        dbg = {
            "dbg_qk": nc.dram_tensor("dbg_qk", [512, S], BF, kind="ExternalOutput").ap(),
            "dbg_v": nc.dram_tensor("dbg_v", [128, NKT * 4 * 65], BF, kind="ExternalOutput").ap(),
            "dbg_att": nc.dram_tensor("dbg_att", [256, S], BF, kind="ExternalOutput").ap(),
        }

    with tile.TileContext(nc) as tc:
        build_tile_program(tc, hsT, wqkT, wvT, woT, cos2T, ssin2T, maskD, outT, dbg)
    nc.compile()
    return nc


def build_tile_program(tc, hsT, wqkT, wvT, woT, cos2T, ssin2T, maskD, outT, dbg=None):
    nc = tc.nc
    with ExitStack() as ctx:
        const = ctx.enter_context(tc.tile_pool(name="const", bufs=1))
        persist = ctx.enter_context(tc.tile_pool(name="persist", bufs=1))
        work = ctx.enter_context(tc.tile_pool(name="work", bufs=3))
        posbp = ctx.enter_context(tc.tile_pool(name="posbp", bufs=12))
        expp = ctx.enter_context(tc.tile_pool(name="expp", bufs=8))
        small = ctx.enter_context(tc.tile_pool(name="small", bufs=3))
        ps_main = ctx.enter_context(tc.tile_pool(name="ps_main", bufs=4, space="PSUM"))
        ps_sc = ctx.enter_context(tc.tile_pool(name="ps_sc", bufs=2, space="PSUM"))

        # ---- inputs to SBUF (one DMA instruction per tensor / hs chunk) ----
        wqk_sb = const.tile([128, KHID, 512], BF, name="wqk_sb")
        nc.scalar.dma_start(wqk_sb[:, 0:4, :],
                            wqkT[0:512, :].rearrange("(k p) m -> p k m", p=128))
        hs_sb = const.tile([128, KHID, S], BF, name="hs_sb")

        def load_hs_chunk(t):
            csl = slice(t * QC, (t + 1) * QC)
            nc.sync.dma_start(
                hs_sb[:, :, csl],
                hsT[:, csl].rearrange("(k p) s -> p k s", p=128),
            )

        nc.sync.dma_start(hs_sb[:, 0:4, 0:QC],
                          hsT[0:512, 0:QC].rearrange("(k p) s -> p k s", p=128))
        nc.gpsimd.dma_start(wqk_sb[:, 4:8, :],
                            wqkT[512:1024, :].rearrange("(k p) m -> p k m", p=128))
        nc.sync.dma_start(hs_sb[:, 4:8, 0:QC],
                          hsT[512:1024, 0:QC].rearrange("(k p) s -> p k s", p=128))
        cos_sb = const.tile([128, S], BF, name="cos_sb")
        nc.sync.dma_start(cos_sb[:], cos2T[:])
        load_hs_chunk(1)
        ssin_sb = const.tile([128, S], BF, name="ssin_sb")
        nc.sync.dma_start(ssin_sb[:], ssin2T[:])
        tri_sb = const.tile([128, 2, 128], BF, name="tri_sb")
        nc.sync.dma_start(tri_sb[:], maskD.rearrange("p (r c) -> p r c", r=2))
        load_hs_chunk(2)
        load_hs_chunk(3)
        wv_sb = const.tile([128, KHID, 256], BF, name="wv_sb")
        nc.sync.dma_start(wv_sb[:], wvT.rearrange("(k p) m -> p k m", p=128))
        wo_sb = const.tile([128, 2, HID], BF, name="wo_sb")
        nc.sync.dma_start(wo_sb[:], woT.rearrange("(k p) m -> p k m", p=128))

        qkT = persist.tile([128, 4, S], BF, name="qkT")
        l_tiles = persist.tile([64, 8, QC], FP, name="l_tiles")
        nc.vector.memset(l_tiles[:], 1.0)
        v_sb = persist.tile([128, NKT, 4 * 65], BF, name="v_sb2")
        att_sb = persist.tile([128, 2, S], BF, name="att_sb2")
        nc.vector.memset(
            v_sb.rearrange("p t (h c) -> p t h c", c=65)[:, :, :, 64:65], 1.0
        )

        def proj_chunk(rb, t):
            csl = slice(t * QC, (t + 1) * QC)
            ps = ps_main.tile([128, QC], FP, name="ps_qk", tag="ps")
            for kk in range(KHID):
                nc.tensor.matmul(
                    ps[:],
                    wqk_sb[:, kk, rb * 128:(rb + 1) * 128],
                    hs_sb[:, kk, csl],
                    start=(kk == 0),
                    stop=(kk == KHID - 1),
                )
            x = work.tile([128, QC], BF, name="x_rope", tag="xrope")
            if rb in (1, 3):
                nc.vector.tensor_copy(x[:], ps[:])
            else:
                nc.scalar.copy(x[:], ps[:])
            xs = work.tile([128, QC], BF, name="xs_rope", tag="xsrope")
            for q in range(4):
                src = (q // 2) * 64 + (32 if q % 2 == 0 else 0)
                dst = (q // 2) * 64 + (0 if q % 2 == 0 else 32)
                nc.vector.tensor_copy(xs[dst:dst + 32, :], x[src:src + 32, :])
            t1 = work.tile([128, QC], BF, name="t1_rope", tag="t1rope")
            nc.vector.tensor_mul(t1[:], x[:], cos_sb[:, csl])
            nc.vector.tensor_mul(xs[:], xs[:], ssin_sb[:, csl])
            nc.vector.tensor_add(qkT[:, rb, csl], t1[:], xs[:])

        def v_proj(tt):
            psv = ps_main.tile([128, 256], FP, name="ps_v", tag="ps")
            for kk in range(KHID):
                nc.tensor.matmul(
                    psv[:],
                    hs_sb[:, kk, tt * 128:(tt + 1) * 128],
                    wv_sb[:, kk, :],
                    start=(kk == 0),
                    stop=(kk == KHID - 1),
                )
            nc.vector.tensor_copy(
                v_sb[:, tt, :].rearrange("p (h c) -> p h c", c=65)[:, :, 0:64],
                psv[:].rearrange("p (h c) -> p h c", c=64),
            )

        def attention_unit(pair, qi, l_pair):
            """scores^T -> exp -> PV for heads (2*pair, 2*pair+1).

            Returns (po_sb0, po_sb1); denominators land in l_pair rows 0 and 32."""
            qsl = slice(qi * QC, (qi + 1) * QC)
            nki = 4 * qi + 4
            po0 = ps_main.tile([65, QC], FP, name="po0", tag="ps")
            po1 = ps_main.tile([65, QC], FP, name="po1", tag="ps")
            for ki in range(nki):
                ksl = slice(ki * KT, (ki + 1) * KT)
                psc = ps_sc.tile([128, 2, QC], FP, name="psc", tag="sc")
                nc.tensor.matmul(
                    psc[:, 0, :], qkT[0:64, 2 + pair, ksl], qkT[0:64, pair, qsl],
                    start=True, stop=True,
                )
                nc.tensor.matmul(
                    psc[:, 1, :], qkT[64:128, 2 + pair, ksl], qkT[64:128, pair, qsl],
                    start=True, stop=True,
                )
                e = expp.tile([128, 2, QC], BF, name="e", tag="exp")
                j = ki - 4 * qi
                lo = 0 if j < 0 else 128 * j  # first live q column in this chunk
                nc.scalar.activation(
                    e[:, :, lo:QC], psc[:, :, lo:QC], EXP, scale=0.125
                )
                if j >= 0:
                    nc.vector.tensor_mul(
                        e[:, :, lo:lo + 128], e[:, :, lo:lo + 128], tri_sb[:]
                    )
                h0 = 2 * pair
                h1 = 2 * pair + 1
                nc.tensor.matmul(
                    po0[:, lo:QC], v_sb[:, ki, h0 * 65:(h0 + 1) * 65], e[:, 0, lo:QC],
                    start=(ki == 0), stop=(ki == nki - 1),
                )
                nc.tensor.matmul(
                    po1[:, lo:QC], v_sb[:, ki, h1 * 65:(h1 + 1) * 65], e[:, 1, lo:QC],
                    start=(ki == 0), stop=(ki == nki - 1),
                )
            # free PSUM fast: numerators to SBUF bf16, denominator rows to l_all
            po_sb0 = posbp.tile([64, QC], BF, name="po_sb0", tag="posb")
            po_sb1 = posbp.tile([64, QC], BF, name="po_sb1", tag="posb")
            nc.vector.tensor_copy(po_sb0[:], po0[0:64, :])
            nc.vector.tensor_copy(po_sb1[:], po1[0:64, :])
            nc.vector.tensor_copy(l_pair[0:1, :], po0[64:65, :])
            nc.vector.tensor_copy(l_pair[32:33, :], po1[64:65, :])
            return po_sb0, po_sb1

        def division(pair, qi, l_pair, po_sb0, po_sb1):
            qsl = slice(qi * QC, (qi + 1) * QC)
            rl = small.tile([64, QC], FP, name="rl", tag="rl", bufs=4)
            scr = small.tile([64, QC], FP, name="scr", tag="scr", bufs=3)
            nc.vector.reciprocal_approx_accurate(out=rl[:], in_=l_pair[:], scratch=scr[:])
            for sub, posb in enumerate([po_sb0, po_sb1]):
                if sub == 0:
                    src = rl[0:1, :]
                else:
                    rlrow = small.tile([1, QC], FP, name="rlrow", tag="rlrow", bufs=4)
                    nc.vector.tensor_copy(rlrow[:], rl[32:33, :])
                    src = rlrow[:]
                rb_ = small.tile([64, QC], FP, name="rb_", tag="rbb", bufs=4)
                nc.gpsimd.partition_broadcast(rb_[:], src)
                nc.vector.tensor_mul(
                    att_sb[sub * 64:(sub + 1) * 64, pair, qsl], posb[:], rb_[:]
                )

        def oproj(qi, last=False):
            qsl = slice(qi * QC, (qi + 1) * QC)
            for half in range(2):
                ow = work.tile([128, 4, QC], BF, name="ow", tag="ow")
                for oi in range(4):
                    ot = half * 4 + oi
                    pw = ps_main.tile([128, QC], FP, name="pw", tag="ps")
                    for p in range(2):
                        nc.tensor.matmul(
                            pw[:],
                            wo_sb[:, p, ot * 128:(ot + 1) * 128],
                            att_sb[:, p, qsl],
                            start=(p == 0),
                            stop=(p == 1),
                        )
                    if ot % 2 == 0:
                        nc.vector.tensor_copy(ow[:, oi, :], pw[:])
                    else:
                        nc.scalar.copy(ow[:, oi, :], pw[:])
                nc.gpsimd.dma_start(
                    outT[half * 512:(half + 1) * 512, qsl].rearrange(
                        "(o p) s -> p o s", p=128),
                    ow[:],
                )

        # emission: pair0 projections up front; pair1 projections, v, attention
        # and (one chunk behind) o_proj interleaved per q chunk so the PE always
        # has ready fill work while ACT grinds through the exps.
        for t in range(NQC):
            proj_chunk(0, t)
            proj_chunk(2, t)
        qi_order = [1, 2, 3, 0]
        loaded = 0
        prev = None
        for qi in qi_order:
            while loaded <= min(qi + 1, NQC - 1):
                proj_chunk(1, loaded)
                proj_chunk(3, loaded)
                for tt in range(4 * loaded, 4 * loaded + 4):
                    v_proj(tt)
                loaded += 1
            l0 = l_tiles[:, 2 * qi, :]
            pa = attention_unit(0, qi, l0)
            division(0, qi, l0, *pa)
            l1 = l_tiles[:, 2 * qi + 1, :]
            pb = attention_unit(1, qi, l1)
            if prev is not None:
                oproj(prev)
            division(1, qi, l1, *pb)
            prev = qi
        oproj(prev, last=True)

        if dbg is not None:
            for rb in range(4):
                nc.sync.dma_start(dbg["dbg_qk"][rb * 128:(rb + 1) * 128, :], qkT[:, rb, :])
            nc.sync.dma_start(dbg["dbg_v"][:], v_sb.rearrange("p t c -> p (t c)"))
            for p in range(2):
                nc.sync.dma_start(dbg["dbg_att"][p * 128:(p + 1) * 128, :], att_sb[:, p, :])


# ---------- host-side shard preparation ----------

def make_core_inputs(hidden_states, cos, sin, w_qkv, w_o):
    """Returns list of 8 in_maps (numpy, bf16 where needed)."""
    bf = ml_dtypes.bfloat16
    hs = np.asarray(hidden_states, np.float32)
    cos = np.asarray(cos, np.float32)
    sin = np.asarray(sin, np.float32)
    w_qkv = np.asarray(w_qkv, np.float32)
    w_o = np.asarray(w_o, np.float32)

    cosT = cos.T
    sinT = sin.T
    cos2T = np.concatenate([cosT, cosT], 0).astype(bf)
    ssinT = np.concatenate([-sinT[0:32], sinT[32:64]], 0)
    ssin2T = np.concatenate([ssinT, ssinT], 0).astype(bf)

    kp = np.arange(128)[:, None]
    cc = np.arange(128)[None, :]
    tri = (kp <= cc).astype(bf)
    maskD = np.concatenate([tri, tri], axis=1)

    in_maps = []
    for c in range(8):
        b, g = divmod(c, 4)
        heads = range(4 * g, 4 * g + 4)
        hsT = np.ascontiguousarray(hs[b].T).astype(bf)
        wq = np.concatenate([w_qkv[h * 64:(h + 1) * 64] for h in heads], 0)
        wk = np.concatenate([w_qkv[HID + h * 64:HID + (h + 1) * 64] for h in heads], 0)
        wv = np.concatenate([w_qkv[2 * HID + h * 64:2 * HID + (h + 1) * 64] for h in heads], 0)
        wqkT = np.ascontiguousarray(np.concatenate([wq, wk], 0).T).astype(bf)
        wvT = np.ascontiguousarray(wv.T).astype(bf)
        woT = np.ascontiguousarray(
            np.concatenate([w_o[:, h * 64:(h + 1) * 64] for h in heads], 1).T
        ).astype(bf)
        in_maps.append({
            "hsT": hsT, "wqkT": wqkT, "wvT": wvT, "woT": woT,
            "cos2T": cos2T, "ssin2T": ssin2T, "maskD": maskD,
        })
    return in_maps


def unshard(outTs):
    out = np.zeros((B, S, HID), np.float32)
    for c, oT in enumerate(outTs):
        out[c // 4] += oT.T.astype(np.float32)
    return out


# ---------- standalone kernel entry ----------

from concourse.bass_utils import run_bass_kernel_spmd

_CACHED_NC = None


def get_program():
    global _CACHED_NC
    if _CACHED_NC is None:
        _CACHED_NC = build_program()
    return _CACHED_NC


def run(inputs, trace=False):
    nc = get_program()
    in_maps = make_core_inputs(**inputs)
    res = run_bass_kernel_spmd(nc, in_maps, core_ids=list(range(8)), trace=trace)
    out = np.zeros((B, S, HID), np.float32)
    for c, r in enumerate(res.results):
        out[c // 4] += r["outT"].T.astype(np.float32)
    return out, res


def kernel(**inputs):
    out, _ = run(inputs, trace=False)
    return out
